# revision 1
# baseline (speedup 1.0000x reference)
"""Distributed causal self-attention for 8 TRN2 NeuronCores.

Sharding: tensor-parallel over heads (2 heads/core, all batches), then an
on-device AllToAll redistributes the attention output from head-sharded to
token-sharded so each core computes a disjoint 1024-token slice of the
output projection.  Host work is only slicing / concatenation.

Layouts (per core g, heads h0=2g, h1=2g+1), all matmuls bf16 with fp32 PSUM:
  qT/kT  [128, BT]    rows 0:64 = head h0 dims, 64:128 = head h1 dims
  v_sb   [128, 130*n] per 128-token chunk: cols [130c:130c+64] = v_h0,
                      col 130c+64 = ones, [130c+65:130c+129] = v_h1,
                      col 130c+129 = ones (ones give the softmax sums)
  S^T    [k, q] in PSUM; exp on ACT (no max-subtraction needed: scores are
         ~N(0,1), |S|<~8 after 1/sqrt(D) scaling, exp never overflows fp32);
         causal mask = bf16 0/1 multiply on GpSimd
  O^T    accumulated in PSUM via matmul(lhsT=v_ext[128,65], rhs=P^T) so
         row 64 = sum_k exp = softmax denominator

Engine budget: PE = matmuls, ACT = exp only, DVE = PSUM->SBUF copies +
normalization, GpSimd = causal masks + collectives, SP = DMA.
The AllToAll is split in two (by 512-token window parity) so the first
collective overlaps the second half of attention and the second overlaps
the first half of the output projection.
"""

import numpy as np

import concourse.bass as bass
import concourse.bacc as bacc
import concourse.mybir as mybir
import concourse.tile as tile
from concourse.bass_utils import run_bass_kernel_spmd

B, T, C = 4, 2048, 1024
H, D = 16, 64
NCORES = 8
HPC = H // NCORES        # heads per core
DH = HPC * D             # 128 attention-output cols per core
P = 128
F32 = mybir.dt.float32
BF16 = mybir.dt.bfloat16
SCALE = 1.0 / np.sqrt(D)


def build_nc(Tb=T, reps=1, stages="ABC", skip_collective=False, bvariant="full", pt_bufs=10, seq_a=True, ps_bufs=3, po_bufs=2, small_bufs=3):
    """Build the SPMD Bass graph (identical on all 8 cores).

    reps > 1 emits the whole pipeline that many times (same buffers, so
    iterations serialize) — used only for steady-state HW timing.
    """
    BT = B * Tb              # total tokens
    NTW = BT // 512          # 512-token windows for QKV
    NQW = Tb // 512          # query windows per batch
    NCH = BT // 128          # 128-token chunks total
    TOKS = BT // NCORES      # tokens per core in the proj stage
    NNW = C // 512           # 512-wide output column windows
    NPH = 2 if Tb >= 2048 else 1   # A2A phase count (split needs 512 | TOKS/NPH)
    HTOK = TOKS // NPH       # tokens per proj phase (A2A split)

    nc = bacc.Bacc(None, target_bir_lowering=False)

    xT_ext = nc.declare_dram_parameter("xT", [C, BT], BF16, isOutput=False)
    wq_ext = nc.declare_dram_parameter("wq", [C, DH], BF16, isOutput=False)
    wk_ext = nc.declare_dram_parameter("wk", [C, DH], BF16, isOutput=False)
    wv_ext = nc.declare_dram_parameter("wv", [C, DH], BF16, isOutput=False)
    wp_ext = nc.declare_dram_parameter("wproj", [C, C], BF16, isOutput=False)
    mk_ext = nc.declare_dram_parameter("masks", [4, P, 512], BF16, isOutput=False)
    id_ext = nc.declare_dram_parameter("ident", [P, P], BF16, isOutput=False)
    y_ext = nc.declare_dram_parameter("y", [TOKS, C], F32, isOutput=True)

    xT_v = xT_ext.rearrange("(c p) t -> p c t", p=P)     # [128, 8, BT]
    wq_v = wq_ext.rearrange("(c p) m -> p c m", p=P)     # [128, 8, 128]
    wk_v = wk_ext.rearrange("(c p) m -> p c m", p=P)
    wv_v = wv_ext.rearrange("(c p) m -> p c m", p=P)
    wp_v = wp_ext.rearrange("(c p) m -> p c m", p=P)     # [128, 8, 1024]
    mk_v = mk_ext.rearrange("j p t -> p j t")            # [128, 4, 512]

    with tile.TileContext(nc, num_cores=NCORES) as tc:
        with (
            tc.tile_pool(name="consts", bufs=1) as consts,
            tc.tile_pool(name="acts", bufs=1) as acts,
            tc.tile_pool(name="xin", bufs=2) as xin,
            tc.tile_pool(name="small", bufs=small_bufs) as small,
            tc.tile_pool(name="ptiles", bufs=pt_bufs) as ptiles,
            tc.tile_pool(name="psum", bufs=1, space="PSUM") as psum,
            tc.tile_pool(name="dram", bufs=1, space="DRAM") as dram,
        ):
            # ---- constants ----
            wq_sb = consts.tile([P, 8, DH], BF16)
            wk_sb = consts.tile([P, 8, DH], BF16)
            wv_sb = consts.tile([P, 8, DH], BF16)
            wp_sb = consts.tile([P, 8, C], BF16)
            mk_sb = consts.tile([P, 4, 512], BF16)
            id_sb = consts.tile([P, P], BF16)
            ones_sb = consts.tile([1, P], BF16)
            nc.gpsimd.dma_start(wq_sb[:], wq_v[:])
            nc.gpsimd.dma_start(wk_sb[:], wk_v[:])
            nc.gpsimd.dma_start(wv_sb[:], wv_v[:])
            nc.gpsimd.dma_start(wp_sb[:], wp_v[:])
            nc.gpsimd.dma_start(mk_sb[:], mk_v[:])
            nc.gpsimd.dma_start(id_sb[:], id_ext[:])
            nc.vector.memset(ones_sb[:], 1.0)

            # ---- persistent activations ----
            qT_sb = acts.tile([P, BT], BF16)
            kT_sb = acts.tile([P, BT], BF16)
            v_sb = acts.tile([P, 130 * NCH], BF16)
            nc.vector.memset(v_sb[:], 1.0)  # bakes in the ones columns

            a2a_in = [dram.tile([NCORES, P, HTOK], BF16, name=f"a2ain{p}",
                                tag=f"a2ain{p}") for p in range(NPH)]
            a2a_out = [dram.tile([NCORES, P, HTOK], BF16, name=f"a2aout{p}",
                                 tag=f"a2aout{p}") for p in range(NPH)]

            for rep in range(reps):
                # ================= Stage A: QKV projection =================
                for tw in range(NTW):
                    xw = xin.tile([P, 8, 512], BF16, tag="xw")
                    nc.sync.dma_start(xw[:], xT_v[:, :, 512 * tw : 512 * (tw + 1)])
                    if seq_a:
                        pq = psum.tile([P, 512], F32, tag="stA", bufs=2)
                        for cc in range(8):
                            nc.tensor.matmul(pq[:], wq_sb[:, cc, :], xw[:, cc, :],
                                             start=(cc == 0), stop=(cc == 7))
                        nc.vector.tensor_copy(qT_sb[:, 512 * tw : 512 * (tw + 1)], pq[:])
                        pk = psum.tile([P, 512], F32, tag="stA", bufs=2)
                        for cc in range(8):
                            nc.tensor.matmul(pk[:], wk_sb[:, cc, :], xw[:, cc, :],
                                             start=(cc == 0), stop=(cc == 7))
                        nc.vector.tensor_copy(kT_sb[:, 512 * tw : 512 * (tw + 1)], pk[:])
                        pvT = psum.tile([P, 512], F32, tag="stA", bufs=2)
                        for cc in range(8):
                            nc.tensor.matmul(pvT[:], wv_sb[:, cc, :], xw[:, cc, :],
                                             start=(cc == 0), stop=(cc == 7))
                        vT_tmp = small.tile([P, 512], BF16, tag="vT")
                        nc.vector.tensor_copy(vT_tmp[:], pvT[:])
                    else:
                        pq = psum.tile([P, 512], F32, tag="pq", bufs=1)
                        pk = psum.tile([P, 512], F32, tag="pk", bufs=1)
                        pvT = psum.tile([P, 512], F32, tag="pvT", bufs=1)
                        for cc in range(8):
                            st, sp = (cc == 0), (cc == 7)
                            rhs = xw[:, cc, :]
                            nc.tensor.matmul(pq[:], wq_sb[:, cc, :], rhs,
                                             start=st, stop=sp)
                            nc.tensor.matmul(pk[:], wk_sb[:, cc, :], rhs,
                                             start=st, stop=sp)
                            nc.tensor.matmul(pvT[:], wv_sb[:, cc, :], rhs,
                                             start=st, stop=sp)
                        nc.vector.tensor_copy(qT_sb[:, 512 * tw : 512 * (tw + 1)], pq[:])
                        nc.vector.tensor_copy(kT_sb[:, 512 * tw : 512 * (tw + 1)], pk[:])
                        vT_tmp = small.tile([P, 512], BF16, tag="vT")
                        nc.vector.tensor_copy(vT_tmp[:], pvT[:])
                    for j in range(4):
                        pv = psum.tile([P, P], BF16, tag="aux", bufs=1)
                        nc.tensor.transpose(pv[:], vT_tmp[:, P * j : P * (j + 1)],
                                            id_sb[:])
                        gc = 4 * tw + j
                        # both heads in one strided copy: cols {0:64, 65:129}
                        dst = v_sb[:, 130 * gc : 130 * gc + 130].rearrange(
                            "p (h d) -> p h d", h=2, d=65)[:, :, 0:64]
                        src = pv[:, :].rearrange("p (h d) -> p h d", h=2)
                        nc.vector.tensor_copy(dst, src)

                # ================= Stage B: causal attention =================
                # qw-major order so each A2A phase's inputs finish early.
                if "B" not in stages:
                    continue
                phases = [(p, [qw for qw in range(NQW)
                               if ((512 * qw) % TOKS) // HTOK == p])
                          for p in range(NPH)]
                for phase, qws in phases:
                    for qw in qws:
                        for b in range(B):
                            for lh in range(HPC):
                                hs = 64 * lh
                                q0 = Tb * b + 512 * qw
                                kmax = 4 * qw + 4
                                po = psum.tile([P, 512], F32, tag="po", bufs=po_bufs)
                                for kc in range(kmax):
                                    k0 = Tb * b + P * kc
                                    j = kc - 4 * qw
                                    # diagonal tiles: columns [0 : 128j] are
                                    # fully causal-masked -> skip them entirely
                                    c0 = max(0, j) * P
                                    ps = psum.tile([P, 512], F32, tag="ps", bufs=ps_bufs)
                                    nc.tensor.matmul(
                                        ps[:, c0:512],
                                        kT_sb[hs : hs + 64, k0 : k0 + P],
                                        qT_sb[hs : hs + 64, q0 + c0 : q0 + 512],
                                        start=True, stop=True)
                                    if bvariant == "sonly":
                                        continue
                                    pT = ptiles.tile([P, 512], BF16, tag="pT")
                                    func = (mybir.ActivationFunctionType.Copy
                                            if bvariant == "noexp" else
                                            mybir.ActivationFunctionType.Exp)
                                    nc.scalar.activation(
                                        pT[:, c0:512], ps[:, c0:512],
                                        func, scale=float(SCALE))
                                    if j >= 0:
                                        nc.vector.tensor_mul(pT[:, c0:512],
                                                             pT[:, c0:512],
                                                             mk_sb[:, j, c0:512])
                                    if bvariant == "nopv":
                                        continue
                                    gc = (Tb // 128) * b + kc
                                    nc.tensor.matmul(
                                        po[0:65, c0:512],
                                        v_sb[:, 130 * gc + 65 * lh
                                             : 130 * gc + 65 * lh + 65],
                                        pT[:, c0:512],
                                        start=(kc == 0), stop=(kc == kmax - 1),
                                        skip_group_check=True)
                                # normalize: rows 0:64 / row 64
                                if bvariant in ("sonly", "nopv", "nonorm"):
                                    continue
                                rec = small.tile([1, 512], F32, tag="rec")
                                nc.vector.reciprocal(rec[:], po[64:65, :])
                                rec_bf = small.tile([1, 512], BF16, tag="recb")
                                nc.vector.tensor_copy(rec_bf[:], rec[:])
                                pb = psum.tile([P, 512], F32, tag="aux", bufs=1)
                                nc.tensor.matmul(pb[:], ones_sb[:], rec_bf[:],
                                                 start=True, stop=True)
                                rb = small.tile([P, 512], F32, tag="rb")
                                nc.vector.tensor_copy(rb[:], pb[:])
                                ao = small.tile([64, 512], BF16, tag="ao")
                                nc.vector.tensor_mul(ao[:], po[0:64, :],
                                                     rb[0:64, :])
                                # dest: global tokens [q0 : q0+512], split on
                                # rank/phase chunk boundaries
                                seg = 0
                                while seg < 512:
                                    t = q0 + seg
                                    r, off = t // TOKS, t % TOKS
                                    ph, offp = off // HTOK, off % HTOK
                                    ln = min(512 - seg, HTOK - offp)
                                    nc.sync.dma_start(
                                        a2a_in[ph][r, hs : hs + 64,
                                                   offp : offp + ln],
                                        ao[:, seg : seg + ln])
                                    seg += ln
                    # fire this phase's AllToAll
                    if skip_collective:
                        continue
                    nc.gpsimd.collective_compute(
                        "AllToAll",
                        mybir.AluOpType.bypass,
                        replica_groups=[list(range(NCORES))],
                        ins=[a2a_in[phase].opt()],
                        outs=[a2a_out[phase].opt()],
                    )

                # ================= Stage C: output projection =================
                if "C" not in stages:
                    continue
                for phase in range(NPH):
                    ga = acts.tile([P, 8, HTOK], BF16, name=f"ga{phase}",
                                   tag=f"ga{phase}")
                    nc.sync.dma_start(ga[:],
                                      a2a_out[phase].rearrange("j p t -> p j t"))
                    for tc2 in range(HTOK // P):
                        for nw in range(NNW):
                            py = psum.tile([P, 512], F32, tag="ps", bufs=ps_bufs)
                            for cc in range(8):
                                nc.tensor.matmul(
                                    py[:],
                                    ga[:, cc, P * tc2 : P * (tc2 + 1)],
                                    wp_sb[:, cc, 512 * nw : 512 * (nw + 1)],
                                    start=(cc == 0), stop=(cc == 7))
                            ys = small.tile([P, 512], F32, tag="ys")
                            nc.vector.tensor_copy(ys[:], py[:])
                            nc.sync.dma_start(
                                y_ext[HTOK * phase + P * tc2
                                      : HTOK * phase + P * (tc2 + 1),
                                      512 * nw : 512 * (nw + 1)],
                                ys[:])

    nc.finalize()
    return nc


def _host_inputs(x, w_attn, w_proj, Tb=T):
    import ml_dtypes
    bf16 = ml_dtypes.bfloat16
    BT = B * Tb
    xT = np.ascontiguousarray(x.reshape(BT, C).T).astype(bf16)
    wproj_bf = np.ascontiguousarray(w_proj).astype(bf16)
    jj = np.arange(4)[:, None, None]
    rr = np.arange(P)[None, :, None]
    ccols = np.arange(512)[None, None, :]
    masks = (P * jj + rr <= ccols).astype(bf16)
    ident = np.eye(P).astype(bf16)
    in_maps = []
    for g in range(NCORES):
        in_maps.append({
            "xT": xT,
            "wq": np.ascontiguousarray(w_attn[:, DH * g : DH * (g + 1)]).astype(bf16),
            "wk": np.ascontiguousarray(w_attn[:, C + DH * g : C + DH * (g + 1)]).astype(bf16),
            "wv": np.ascontiguousarray(w_attn[:, 2 * C + DH * g : 2 * C + DH * (g + 1)]).astype(bf16),
            "wproj": wproj_bf,
            "masks": masks,
            "ident": ident,
        })
    return in_maps


_NC_CACHE = {}


def kernel(x, w_attn, w_proj):
    x = np.asarray(x)
    w_attn = np.asarray(w_attn)
    w_proj = np.asarray(w_proj)
    if T not in _NC_CACHE:
        _NC_CACHE[T] = build_nc(T)
    nc = _NC_CACHE[T]
    in_maps = _host_inputs(x, w_attn, w_proj, T)
    res = run_bass_kernel_spmd(nc, in_maps, core_ids=list(range(NCORES)))
    y = np.concatenate([res.results[g]["y"] for g in range(NCORES)], axis=0)
    return y.reshape(B, T, C).astype(np.float32)



# revision 32
# speedup vs baseline: 1.2515x; 1.2515x over previous
"""Distributed causal self-attention for 8 TRN2 NeuronCores (v2).

Sharding: tensor-parallel over heads (2 heads/core, all batches); an
AllToAll (split in 2 phases) redistributes the attention output from
head-sharded to token-sharded for the output projection.

v2 structure (per core, all matmuls bf16, fp32 PSUM):
  - qw-major rounds interleaving QKV projection (stage A) with attention
    (stage B): round r computes scores/exp/PV for query window r of all
    batches while the PE also runs stage-A matmuls for window r+1, so the
    ACT engine's exp stream hides under PE work.
  - scores for the two heads are emitted back-to-back as 64-row PE tiles
    (lhsT/rhs base partitions 0 and 64) -> they run concurrently in the
    PE array (row tiling).
  - per key-chunk PSUM tile [128, 1024]: cols 0:512 head0, 512:1024
    head1; ONE exp instruction per chunk with a rank-3 AP that skips the
    causally-dead prefix of diagonal chunks.  Causal triangle masked by a
    bf16 0/1 multiply on the [128,128] diagonal tiles only.
  - PV accumulates O^T (rows 0:64) and the softmax denominator (row 64,
    via a ones column baked into v_sb) in PSUM; the UNNORMALIZED output
    plus denominator rows travel through the AllToAll ([8, 130, 512]
    payload) and normalization happens on the receive side: one fast
    reciprocal + gpsimd partition-broadcasts + one big DVE multiply per
    phase.
"""

import numpy as np

import concourse.bass as bass
import concourse.bacc as bacc
import concourse.mybir as mybir
import concourse.tile as tile
from concourse.bass_utils import run_bass_kernel_spmd
from concourse.dve_ops import RECIPROCAL_APPROX_FAST, RECIP_APPROX_FAST_CONSTS

B, T, C = 4, 2048, 1024
H, D = 16, 64
NCORES = 8
HPC = H // NCORES        # heads per core
DH = HPC * D             # 128 qkv cols per core
P = 128
F32 = mybir.dt.float32
BF16 = mybir.dt.bfloat16
SCALE = 1.0 / np.sqrt(D)


def build_nc(Tb=T, reps=1, debug=False, do_norm=True):
    BT = B * Tb              # total tokens
    NTW = BT // 512          # 512-token windows (stage A units)
    NQW = Tb // 512          # query windows per batch
    NCH = BT // 128          # 128-token chunks total
    TOKS = BT // NCORES      # tokens per core in the proj stage
    NNW = C // 512           # output column windows
    NPH = 2 if Tb >= 2048 else 1
    HTOK = TOKS // NPH       # tokens per proj phase

    nc = bacc.Bacc(None, target_bir_lowering=False)

    xT_ext = nc.declare_dram_parameter("xT", [C, BT], BF16, isOutput=False)
    wq_ext = nc.declare_dram_parameter("wq", [C, DH], BF16, isOutput=False)
    wk_ext = nc.declare_dram_parameter("wk", [C, DH], BF16, isOutput=False)
    wv_ext = nc.declare_dram_parameter("wv", [C, DH], BF16, isOutput=False)
    wp_ext = nc.declare_dram_parameter("wproj", [C, C], BF16, isOutput=False)
    mk_ext = nc.declare_dram_parameter("masks", [P, 256], BF16, isOutput=False)
    sel_ext = nc.declare_dram_parameter("sel", [8, 16, P], BF16, isOutput=False)
    id_ext = nc.declare_dram_parameter("ident", [P, P], BF16, isOutput=False)
    y_ext = nc.declare_dram_parameter("y", [TOKS, C], F32, isOutput=True)
    dbg_ext = (nc.declare_dram_parameter("dbg", [NPH, 2, 16, HTOK], F32,
                                         isOutput=True) if debug else None)
    dbg2_ext = (nc.declare_dram_parameter("dbg2", [3, P, 1024], F32,
                                          isOutput=True) if debug else None)
    dbg3_ext = (nc.declare_dram_parameter("dbg3", [2, 130, 512], BF16,
                                          isOutput=True) if debug else None)
    dbg4_ext = (nc.declare_dram_parameter("dbg4", [P, 8, HTOK], BF16,
                                          isOutput=True) if debug else None)

    xT_v = xT_ext.rearrange("(c p) t -> p c t", p=P)     # [128, 8, BT]
    wq_v = wq_ext.rearrange("(c p) m -> p c m", p=P)
    wk_v = wk_ext.rearrange("(c p) m -> p c m", p=P)
    wv_v = wv_ext.rearrange("(c p) m -> p c m", p=P)
    wp_v = wp_ext.rearrange("(c p) m -> p c m", p=P)     # [128, 8, 1024]

    with tile.TileContext(nc, num_cores=NCORES) as tc:
        with (
            tc.tile_pool(name="consts", bufs=1) as consts,
            tc.tile_pool(name="acts", bufs=1) as acts,
            tc.tile_pool(name="xin", bufs=2) as xin,
            tc.tile_pool(name="small", bufs=4) as small,
            tc.tile_pool(name="ptiles", bufs=(16 if debug else 18)) as ptiles,
            tc.tile_pool(name="psum", bufs=1, space="PSUM") as psum,
            tc.tile_pool(name="dram", bufs=1, space="DRAM") as dram,
        ):
            # ---- constants ----
            wq_sb = consts.tile([P, 8, DH], BF16)
            wk_sb = consts.tile([P, 8, DH], BF16)
            wv_sb = consts.tile([P, 8, DH], BF16)
            wp_sb = consts.tile([P, 8, C], BF16)
            mk_sb = consts.tile([P, 256], BF16)   # [tri | tri]
            id_sb = consts.tile([P, P], BF16)
            sel_sb = consts.tile([8, 16, P], BF16)
            nc.gpsimd.dma_start(sel_sb[:], sel_ext[:])
            nc.gpsimd.dma_start(wq_sb[:], wq_v[:])
            nc.gpsimd.dma_start(wk_sb[:], wk_v[:])
            nc.gpsimd.dma_start(wv_sb[:], wv_v[:])
            nc.gpsimd.dma_start(wp_sb[:], wp_v[:])
            nc.gpsimd.dma_start(mk_sb[:], mk_ext[:])
            nc.gpsimd.dma_start(id_sb[:], id_ext[:])

            # ---- persistent activations ----
            qT_sb = acts.tile([P, BT], BF16)
            kT_sb = acts.tile([P, BT], BF16)
            v_sb = acts.tile([P, 130 * NCH], BF16)
            nc.vector.memset(v_sb[:], 1.0)  # bakes in the ones columns

            a2a_in = [dram.tile([NCORES, 130, HTOK], BF16, name=f"a2ain{p}",
                                tag=f"a2ain{p}") for p in range(NPH)]
            a2a_out = [dram.tile([NCORES, 130, HTOK], BF16, name=f"a2aout{p}",
                                 tag=f"a2aout{p}") for p in range(NPH)]

            def stage_a(tw):
                """QKV projection for one 512-token window; emits matmul
                groups interleaved with copies."""
                xw = xin.tile([P, 8, 512], BF16, tag="xw")
                nc.sync.dma_start(xw[:], xT_v[:, :, 512 * tw: 512 * (tw + 1)])
                pq = psum.tile([P, 512], F32, tag="stA", bufs=2)
                for cc in range(8):
                    nc.tensor.matmul(pq[:], wq_sb[:, cc, :], xw[:, cc, :],
                                     start=(cc == 0), stop=(cc == 7))
                nc.vector.tensor_copy(qT_sb[:, 512 * tw: 512 * (tw + 1)], pq[:])
                pk = psum.tile([P, 512], F32, tag="stA", bufs=2)
                for cc in range(8):
                    nc.tensor.matmul(pk[:], wk_sb[:, cc, :], xw[:, cc, :],
                                     start=(cc == 0), stop=(cc == 7))
                nc.vector.tensor_copy(kT_sb[:, 512 * tw: 512 * (tw + 1)], pk[:])
                pvT = psum.tile([P, 512], F32, tag="stA", bufs=2)
                for cc in range(8):
                    nc.tensor.matmul(pvT[:], wv_sb[:, cc, :], xw[:, cc, :],
                                     start=(cc == 0), stop=(cc == 7))
                vT_tmp = small.tile([P, 512], BF16, tag="vT")
                nc.vector.tensor_copy(vT_tmp[:], pvT[:])
                for j in range(4):
                    pv = psum.tile([P, P], BF16, tag="stA", bufs=2)
                    nc.tensor.transpose(pv[:], vT_tmp[:, P * j: P * (j + 1)],
                                        id_sb[:])
                    gc = 4 * tw + j
                    dst = v_sb[:, 130 * gc: 130 * gc + 130].rearrange(
                        "p (h d) -> p h d", h=2, d=65)[:, :, 0:64]
                    src = pv[:, :].rearrange("p (h d) -> p h d", h=2)
                    nc.vector.tensor_copy(dst, src)

            def proj_norm(phase):
                """Receive side of one AllToAll phase: load + normalize.
                Returns the normalized [P, 8, HTOK] activation tile."""
                ga = acts.tile([P, 8, HTOK], BF16, tag=f"ga{phase}")
                nc.sync.dma_start(
                    ga[:], a2a_out[phase][:, 0:P, :].rearrange("j p t -> p j t"))
                dn = small.tile([8, 2, HTOK], BF16, tag=f"dn{phase}", bufs=1)
                nc.sync.dma_start(dn[:], a2a_out[phase][:, P:P + 2, :])
                dnf32 = small.tile([8, 2, HTOK], F32, tag=f"dnf32{phase}", bufs=1)
                nc.vector.tensor_copy(dnf32[:], dn[:])
                rf32 = small.tile([8, 2, HTOK], F32, tag=f"rf32{phase}", bufs=1)
                cst = RECIP_APPROX_FAST_CONSTS
                nc.vector._custom_dve(RECIPROCAL_APPROX_FAST, out=rf32[:],
                                      in0=dnf32[:], s0=cst["s0"], s1=cst["s1"],
                                      imm2=cst["imm2"])
                rf = small.tile([8, 2, HTOK], BF16, tag=f"rf{phase}", bufs=1)
                nc.vector.tensor_copy(rf[:], rf32[:])
                if debug:
                    nc.sync.dma_start(
                        dbg_ext[phase, 0].rearrange("(j h) t -> j h t", h=2),
                        dnf32[:])
                    nc.sync.dma_start(
                        dbg_ext[phase, 1].rearrange("(j h) t -> j h t", h=2),
                        rf32[:])
                if do_norm:
                    # broadcast 1/den along partitions via K=8 select
                    # matmuls, then scale ga straight from PSUM
                    for j in range(NCORES):
                        pb = psum.tile([P, HTOK], F32, tag="stA", bufs=2)
                        for h in range(HPC):
                            nc.tensor.matmul(pb[:], sel_sb[:, 2 * j + h, :],
                                             rf[:, h, :],
                                             start=(h == 0), stop=(h == 1))
                        nc.vector.tensor_mul(ga[:, j, :], ga[:, j, :], pb[:])
                return ga

            def proj_unit(phase, ga, tc2):
                """One 128-token slice of the output projection."""
                for nw in range(NNW):
                    py = psum.tile([P, 512], F32, tag="stA", bufs=2)
                    for cc in range(8):
                        nc.tensor.matmul(
                            py[:], ga[:, cc, P * tc2: P * (tc2 + 1)],
                            wp_sb[:, cc, 512 * nw: 512 * (nw + 1)],
                            start=(cc == 0), stop=(cc == 7))
                    ys = small.tile([P, 512], F32, tag="ys")
                    nc.vector.tensor_copy(ys[:], py[:])
                    nc.sync.dma_start(
                        y_ext[HTOK * phase + P * tc2: HTOK * phase + P * (tc2 + 1),
                              512 * nw: 512 * (nw + 1)],
                        ys[:])

            def fire_a2a(phase):
                nc.gpsimd.collective_compute(
                    "AllToAll", mybir.AluOpType.bypass,
                    replica_groups=[list(range(NCORES))],
                    ins=[a2a_in[phase].opt()], outs=[a2a_out[phase].opt()])

            for rep in range(reps):
                # prologue: stage A for round 0 (tw = 4b)
                for b in range(B):
                    stage_a(4 * b)
                ga0 = None

                for qw in range(NQW):
                    kmax = 4 * qw + 4
                    for b in range(B):
                        q0 = Tb * b + 512 * qw
                        r = q0 // TOKS
                        ph = (q0 % TOKS) // HTOK

                        def filler():
                            # PE work woven between score chunks so the PE
                            # stays busy while ACT drains the exp backlog
                            nonlocal ga0
                            if qw + 1 < NQW:
                                stage_a(4 * b + qw + 1)
                            elif NPH == 2:
                                if ga0 is None:
                                    ga0 = proj_norm(0)
                                proj_unit(0, ga0, b)

                        # ---- scores + exp for both heads, per chunk ----
                        pts = []
                        for kc in range(kmax):
                            k0 = Tb * b + P * kc
                            j = kc - 4 * qw
                            c0 = max(0, j) * P
                            ps = psum.tile([P, 1024], F32, tag="pair", bufs=2)
                            pt = ptiles.tile([P, 1024], BF16, tag="pT")
                            pts.append((pt, c0))
                            for lh in range(HPC):
                                hs = 64 * lh
                                nc.tensor.matmul(
                                    ps[:, 512 * lh + c0: 512 * lh + 512],
                                    kT_sb[hs: hs + 64, k0: k0 + P],
                                    qT_sb[hs: hs + 64, q0 + c0: q0 + 512],
                                    start=True, stop=True)
                            # one exp for both heads; rank-3 AP skips the
                            # dead prefix of diagonal chunks
                            src = ps[:].rearrange("p (h x) -> p h x", h=2)
                            dst = pt[:].rearrange("p (h x) -> p h x", h=2)
                            nc.scalar.activation(
                                dst[:, :, c0:512], src[:, :, c0:512],
                                mybir.ActivationFunctionType.Exp,
                                scale=float(SCALE))
                            if j >= 0:
                                # causal triangle on the diagonal tile
                                nc.vector.tensor_mul(
                                    dst[:, :, c0: c0 + P],
                                    dst[:, :, c0: c0 + P],
                                    mk_sb[:].rearrange("p (h x) -> p h x", h=2))
                            if debug and qw == 0 and b == 0 and kc == 0:
                                pf = acts.tile([P, 1024], F32, tag="pf")
                                nc.vector.tensor_copy(pf[:], pt[:])
                                nc.sync.dma_start(dbg2_ext[0], pf[:])
                                sf = acts.tile([P, 1024], F32, tag="pf")
                                nc.vector.tensor_copy(sf[:], ps[:])
                                nc.sync.dma_start(dbg2_ext[1], sf[:])
                            if kc == kmax // 2:
                                filler()
                        # ---- PV sweeps: head 0 then head 1 ----
                        for lh in range(HPC):
                            hs = 64 * lh
                            po = psum.tile([P, 512], F32, tag="po", bufs=2)
                            for kc in range(kmax):
                                pt, c0 = pts[kc]
                                gc = (Tb // 128) * b + kc
                                nc.tensor.matmul(
                                    po[0:65, c0:512],
                                    v_sb[:, 130 * gc + 65 * lh:
                                         130 * gc + 65 * lh + 65],
                                    pt[:, 512 * lh + c0: 512 * lh + 512],
                                    start=(kc == 0), stop=(kc == kmax - 1),
                                    skip_group_check=True)
                            oa = small.tile([65, 512], BF16, tag="oa")
                            nc.vector.tensor_copy(oa[:], po[0:65, :])
                            if debug and qw == 0 and b == 0:
                                of = acts.tile([P, 1024], F32, tag="pf")
                                nc.vector.tensor_copy(of[0:65, 512 * lh: 512 * lh + 512],
                                                      po[0:65, :])
                                if lh == 1:
                                    nc.sync.dma_start(dbg2_ext[2], of[:])
                            off = (q0 % TOKS) % HTOK
                            nc.sync.dma_start(
                                a2a_in[ph][r, hs: hs + 64, off: off + 512],
                                oa[0:64, :])
                            nc.sync.dma_start(
                                a2a_in[ph][r, P + lh, off: off + 512],
                                oa[64:65, :])
                    if NPH == 2 and qw == NQW - 2:
                        if debug:
                            nc.sync.dma_start(dbg3_ext[0], a2a_in[0][0])
                        fire_a2a(0)
                        if debug:
                            nc.sync.dma_start(dbg3_ext[1], a2a_out[0][0])
                # final phase: collective + projection
                fire_a2a(NPH - 1)
                ga1 = proj_norm(NPH - 1)
                for tc2 in range(HTOK // P):
                    proj_unit(NPH - 1, ga1, tc2)

    nc.finalize()
    return nc


def _host_inputs(x, w_attn, w_proj, Tb=T):
    import ml_dtypes
    bf16 = ml_dtypes.bfloat16
    BT = B * Tb
    xT = np.ascontiguousarray(x.reshape(BT, C).T).astype(bf16)
    wproj_bf = np.ascontiguousarray(w_proj).astype(bf16)
    rr = np.arange(P)[:, None]
    cc = np.arange(P)[None, :]
    tri = (rr <= cc).astype(bf16)
    masks = np.concatenate([tri, tri], axis=1)   # [128, 256]
    ident = np.eye(P).astype(bf16)
    ii = np.arange(8)[:, None, None]
    ss = np.arange(16)[None, :, None]     # slot = 2j + h
    pp = np.arange(P)[None, None, :]
    sel = ((ii == ss // 2) & (pp // 64 == ss % 2)).astype(bf16)  # [8, 16, 128]
    in_maps = []
    for g in range(NCORES):
        in_maps.append({
            "xT": xT,
            "wq": np.ascontiguousarray(w_attn[:, DH * g: DH * (g + 1)]).astype(bf16),
            "wk": np.ascontiguousarray(w_attn[:, C + DH * g: C + DH * (g + 1)]).astype(bf16),
            "wv": np.ascontiguousarray(w_attn[:, 2 * C + DH * g: 2 * C + DH * (g + 1)]).astype(bf16),
            "wproj": wproj_bf,
            "masks": masks,
            "sel": sel,
            "ident": ident,
        })
    return in_maps


_NC_CACHE = {}


def kernel(x, w_attn, w_proj):
    x = np.asarray(x)
    w_attn = np.asarray(w_attn)
    w_proj = np.asarray(w_proj)
    if T not in _NC_CACHE:
        _NC_CACHE[T] = build_nc(T)
    nc = _NC_CACHE[T]
    in_maps = _host_inputs(x, w_attn, w_proj, T)
    res = run_bass_kernel_spmd(nc, in_maps, core_ids=list(range(NCORES)))
    y = np.concatenate([res.results[g]["y"] for g in range(NCORES)], axis=0)
    return y.reshape(B, T, C).astype(np.float32)
